# revision 21
# baseline (speedup 1.0000x reference)
"""Trainium2 Bass kernel for nn_Autoregression (16-state AR whitening log-prob).

Math: reference computes log_prob[b,k,t] = -0.5*(C*log(2pi) + logdet(Sigma_k)
+ es_k(t)^T Sigma_k^{-1} es_k(t)) with es = causal_conv(x, W, b).  Since
Sigma^{-1} = L^{-T} L^{-1} and es is affine in x, fold L^{-1} into the conv:
W2 = L^{-1} W, b2 = L^{-1} b, then mahalanobis = sum_c conv(x; W2, b2)^2.

fp8 DoubleRow layout (per core, T sharded 8 ways):
taps 0-7 of the 9-tap conv run as fp8e4 DoubleRow matmuls -- the PE array
virtualizes to 256 contraction rows, so 2 DR matmuls replace 4 bf16 ones.
The stationary x window is a 2-plane tile (plane p = x shifted 2p / 2p+1 in
lo/hi rows); DR pair P slices planes at column offset 4P, satisfying the
dual-fp8 LDWEIGHTS rules (even offsets, 16B-aligned plane pitch).  Weights
carry a per-state scale s_k (max |W2_k| -> 96) so e4m3 quantization error
stays relative; the scale divides out in the final tensor_scalar
(per-partition scalar1 = -0.5/s_k^2).  Tap 8 + the bias ride a third fp8
matmul (65 contraction rows: shift-8 x + a ones row).  Per 128-t chunk:
4 DR + 2 normal matmuls accumulate PSUM [128 t, 1024 (state, ch)].  ACT
squares the chunk PSUM -> bf16 SBUF in one op; DVE folds pairs then
segment-reduces to [128, 16] into a [128, 128] tile covering 8 chunks; one
PE transpose per 8 chunks flips to [(chunk, state), 128 t]; DVE applies the
per-state scale/constant; DMA out (host de-interleaves chunk rows).
DMA rings: sync carries wave-0 x pieces + output, gpsimd issues weights +
all later waves (25ns/DMA on its sequencer vs 667 on scalar's).
"""

import os

import numpy as np
import ml_dtypes

import concourse.bass as bass
import concourse.bacc as bacc_mod
import concourse.mybir as mybir
import concourse.tile as tile
from concourse.bass_utils import run_bass_kernel_spmd
from concourse.tile_rust import add_dep_helper

K = 16          # states
C = 64          # channels
T = 65536       # time
AR = 8          # ar order (kernel size AR+1)
NCORES = 8
TLOC = T // NCORES          # 8192 outputs per core
TC = 128                    # outputs per chunk (matmul M)
WAVE = 16                   # chunks per wave (input tile granularity)
WCOLS = TC * WAVE           # 2048 outputs per wave
NW = TLOC // WCOLS          # waves per core
NH = 2                      # psum halves (states 0-7, 8-15)
NPAIR = 2                   # DoubleRow matmuls per half (taps 0-3, 4-7)
GRP = 8                     # chunks per transpose group
XW = WCOLS + 4              # used plane width (chunk 15 pair 1 ends at 2051)
XWP = 2064                  # padded plane pitch (16B aligned)

FP8_DT = mybir.dt.float8e4
SQ_DT = mybir.dt.bfloat16    # squares dtype

_FP8_NP = mybir.dt.np(FP8_DT)

DR = mybir.MatmulPerfMode.DoubleRow

# wave-0 x loads land piecewise so chunk c can start at piece ~c/4
W0_CUTS = [0, 136, 392, 648, 904, 1160, 1416, 1672, 1928, XW]

_CACHE: dict = {}


def _build_program():
    nc = bacc_mod.Bacc()
    f32 = mybir.dt.float32

    # xq plane p: rows 0-63 = x shifted by 2p, rows 64-127 = x shifted 2p+1
    xq = nc.declare_dram_parameter("xq", [128, 2, TLOC + 8], FP8_DT, isOutput=False)
    # tap-8 x (rows 0-63, shift 8) + ones row (bias)
    xeb = nc.declare_dram_parameter("xeb", [128, TLOC], FP8_DT, isOutput=False)
    # fp8 weights: [contraction row, pair, plane, (half, state, ch)]
    wq = nc.declare_dram_parameter("wq", [128, NPAIR, 2, 1024], FP8_DT, isOutput=False)
    # fp8 tail weights: rows 0-63 tap-8, row 64 bias
    w8 = nc.declare_dram_parameter("w8", [128, 1024], FP8_DT, isOutput=False)
    ident = nc.declare_dram_parameter("ident", [128, 128], mybir.dt.float32r, isOutput=False)
    # per-row (16c+k) output affine: col 0 = -0.5/s_k^2, col 1 = -0.5*const_k
    vecs = nc.declare_dram_parameter("vecs", [128, 2], f32, isOutput=False)
    # out[w, 16c'+k, g, m] = log_prob[k, w*2048 + g*1024 + c'*128 + m]
    out = nc.declare_dram_parameter("out", [NW, 128, NH, TC], f32, isOutput=True)

    with tile.TileContext(nc) as tc:
        with (
            tc.tile_pool(name="singles", bufs=1) as singles,
            # one slot per wave: input DMAs never wait (no slot WAR/WAW)
            tc.tile_pool(name="xpool", bufs=NW) as xpool,
            tc.tile_pool(name="sqpool", bufs=6) as sqpool,
            tc.tile_pool(name="sfpool", bufs=6) as sfpool,
            tc.tile_pool(name="mpool", bufs=4) as mpool,
            tc.tile_pool(name="conv_ps", bufs=3, space="PSUM") as conv_ps,
            tc.tile_pool(name="mt_ps", bufs=1, space="PSUM") as mt_ps,
            tc.tile_pool(name="obs_ps", bufs=1, space="PSUM") as obs_ps,
        ):
            # Matmuls must never be the first PE instruction to observe more
            # than one producer semaphore (1-wait ISA slots; bacc's event-sem
            # legalization costs sequencer time).  pe_observe() emits a tiny
            # 2x2 "reader" matmul whose operands come from a single
            # producer's tile; ordering edges pin readers ahead of the next
            # real matmul.
            scratch = obs_ps.tile([2, 128], f32)
            scratch2 = singles.tile([2, 128], SQ_DT)
            nc.vector.memset(scratch2, 0.0)
            pending = []
            obs_after = [None]

            def pe_observe(col):
                i = nc.tensor.matmul(
                    scratch[0:2, 0:2], col, col, start=True, stop=True
                )
                if obs_after[0] is not None:
                    # not earlier than late in the previous wave, or the PE
                    # FIFO head-of-line blocks on a DMA that hasn't landed
                    add_dep_helper(i.ins, obs_after[0].ins, sync=False)
                pending.append(i)

            def _flush(i):
                while pending:
                    add_dep_helper(i.ins, pending.pop().ins, sync=False)
                return i

            def pe_matmul(*args, **kw):
                return _flush(nc.tensor.matmul(*args, **kw))

            # dep-free warmup matmuls: keep the PE busy through the initial
            # input DMAs so HAM un-throttles before real work
            for _ in range(32):
                nc.tensor.matmul(
                    scratch[0:2, 0:128],
                    scratch2[0:2, 0:2],
                    scratch2[0:2, 0:128],
                    start=True,
                    stop=True,
                )

            wq_sb = singles.tile([128, NPAIR, 2, 1024], FP8_DT)
            w8_sb = singles.tile([128, 1024], FP8_DT)
            ident_sb = singles.tile([128, 128], mybir.dt.float32r)
            vec_sb = singles.tile([128, 2], f32)
            out_sb = singles.tile([128, NW * NH, TC], f32)
            xqs, xes = [], []
            # DMA ring plan: the sync HWDGE spreads transfers across many
            # DMA engines -- it carries every x wave (wave-0 piecewise) and
            # the output.  Weights ride the scalar HWDGE in
            # column pieces (idle at startup); gpsimd's single-queue SWDGE only
            # gets the tiny ident/vecs.
            sc_dmas = []
            # h0 halves first so chunk 0's first matmuls unblock earliest;
            # 256-col pieces land on parallel DMA queues (~2.9us each)
            for h in range(NH):
                for P in range(NPAIR):
                    for c0 in range(512 * h, 512 * h + 512, 256):
                        sc_dmas.append(
                            nc.scalar.dma_start(
                                out=wq_sb[:, P, :, c0 : c0 + 256],
                                in_=wq[:, P, :, c0 : c0 + 256],
                            )
                        )
                sc_dmas.append(
                    nc.scalar.dma_start(
                        out=w8_sb[:, 512 * h : 512 * h + 512],
                        in_=w8[:, 512 * h : 512 * h + 512],
                    )
                )
            nc.gpsimd.dma_start(out=ident_sb, in_=ident[:, :])
            nc.gpsimd.dma_start(out=vec_sb, in_=vecs[:, :])
            for w in range(NW):
                xq_sb = xpool.tile([128, 2, XWP], FP8_DT, name="xq")
                xe_sb = xpool.tile([128, WCOLS], FP8_DT, name="xe")
                base = w * WCOLS
                if w == 0:
                    for ci in range(len(W0_CUTS) - 1):
                        lo, hi = W0_CUTS[ci], W0_CUTS[ci + 1]
                        nc.sync.dma_start(
                            out=xq_sb[:, :, lo:hi],
                            in_=xq[:, :, base + lo : base + hi],
                        )
                        he = min(hi, WCOLS)
                        nc.sync.dma_start(
                            out=xe_sb[:, lo:he], in_=xeb[:, base + lo : base + he]
                        )
                else:
                    for lo, hi in ((0, 1026), (1026, XW)):
                        nc.sync.dma_start(
                            out=xq_sb[:, :, lo:hi],
                            in_=xq[:, :, base + lo : base + hi],
                        )
                    nc.sync.dma_start(out=xe_sb, in_=xeb[:, base : base + WCOLS])
                xqs.append(xq_sb)
                xes.append(xe_sb)

            # DVE observer for the vecs DMA (TS struct fits one wait);
            # also pins the vector-ring weight DMAs ahead of DVE compute
            dve_scratch = singles.tile([128, 2], f32)
            nc.vector.tensor_copy(dve_scratch, vec_sb)
            first_sq = [True]

            grp_plan = [(0, 8), (8, 8)]
            grp_last = [(0, 4), (4, 4), (8, 4), (12, 2), (14, 2)]
            for w in range(NW):
                xq_sb = xqs[w]
                xe_sb = xes[w]
                groups = grp_last if w == NW - 1 else grp_plan
                starts = {s: n for s, n in groups}
                ends = {s + n - 1: (s, n) for s, n in groups}
                m8 = None
                for c in range(WAVE):
                    off = c * TC
                    if c in starts:
                        gsz = starts[c]
                        m8 = mpool.tile(
                            [128, 16 * gsz], mybir.dt.float32r, name="m8"
                        )
                        gstart = c
                    cp = c - gstart
                    # one 2-bank psum tile per chunk; halves are bank-aligned
                    ps = conv_ps.tile([128, 1024], f32, name="ps", tag="ps")
                    if w == 0 and c == 0:
                        pe_observe(ident_sb[:, 0:2])
                        pe_observe(xq_sb[:, 0, 0:2])
                        pe_observe(xe_sb[0:2, 0:2])
                    elif w == 0 and c % 2 == 1:
                        lo = W0_CUTS[(c + 1) // 2]
                        pe_observe(xq_sb[:, 0, lo : lo + 2])
                        pe_observe(xe_sb[0:2, lo : lo + 2])
                    # waves 1-3: inputs land long before use; the matmul's
                    # second wait rides its ldweights (bacc legalization)
                    # h-major: finish the h0 bank group before touching h1
                    # weights, matching the h-major weight DMA landing order
                    for h in range(NH):
                        for P in range(NPAIR):
                            lhsT = xq_sb[:, :, off + 4 * P : off + 4 * P + TC]
                            pe_matmul(
                                ps[:, 512 * h : 512 * h + 512],
                                lhsT,
                                wq_sb[:, P, :, 512 * h : 512 * h + 512],
                                start=(P == 0),
                                stop=False,
                                perf_mode=DR,
                            )
                        mm_i = pe_matmul(
                            ps[:, 512 * h : 512 * h + 512],
                            xe_sb[:, off : off + TC],
                            w8_sb[:, 512 * h : 512 * h + 512],
                            start=False,
                            stop=True,
                        )
                    if c == WAVE - 2:
                        obs_after[0] = mm_i
                    # squares on ACT, pair-fold + segmented reduce on DVE.
                    # The last two chunks drain per psum half so the final
                    # reduce lands right after the last matmul.
                    halves = (
                        [(0, 1024)]
                        if not (w == NW - 1 and c >= WAVE - 2)
                        else [(0, 512), (512, 512)]
                    )
                    for lo_h, wd in halves:
                        ng = wd // 64
                        sq = sqpool.tile([128, wd], SQ_DT, name="sq", tag="sq")
                        sq_i = nc.scalar.activation(
                            sq,
                            ps[:, lo_h : lo_h + wd],
                            mybir.ActivationFunctionType.Square,
                        )
                        if first_sq[0]:
                            while sc_dmas:
                                add_dep_helper(
                                    sq_i.ins, sc_dmas.pop().ins, sync=False
                                )
                            first_sq[0] = False
                        sqf = sfpool.tile(
                            [128, wd // 2], SQ_DT, name="sqf", tag="sqf"
                        )
                        sqv = sq.rearrange(
                            "p (g two c) -> p g two c", g=ng, two=2
                        )
                        nc.vector.tensor_tensor(
                            out=sqf.rearrange("p (g c) -> p g c", g=ng),
                            in0=sqv[:, :, 0, :],
                            in1=sqv[:, :, 1, :],
                            op=mybir.AluOpType.add,
                        )
                        with nc.allow_low_precision(
                            reason="float32r shares float32 bits; r-mode only "
                            "affects the PE multiply path"
                        ):
                            nc.vector.tensor_reduce(
                                out=m8[
                                    :,
                                    16 * cp + lo_h // 64 : 16 * cp + lo_h // 64 + ng,
                                ],
                                in_=sqf.rearrange("p (g c) -> p g c", g=ng),
                                axis=mybir.AxisListType.X,
                                op=mybir.AluOpType.add,
                            )
                    if c in ends:
                        gstart, gsz = ends[c]
                        rows = 16 * gsz
                        r0 = 16 * (gstart % GRP)
                        gh = gstart // GRP
                        mt = mt_ps.tile([rows, 128], mybir.dt.float32r, name="mt")
                        _flush(nc.tensor.transpose(mt, m8, ident_sb))
                        # closing 2-chunk groups: pipeline TS with the store
                        # in column halves so the final DMA starts sooner
                        cols = (
                            ((0, 64), (64, 64))
                            if (w == NW - 1 and gsz == 2)
                            else ((0, TC),)
                        )
                        for c0, cw in cols:
                            nc.vector.tensor_scalar(
                                out=out_sb[
                                    r0 : r0 + rows, NH * w + gh, c0 : c0 + cw
                                ],
                                in0=mt[:, c0 : c0 + cw],
                                scalar1=vec_sb[r0 : r0 + rows, 0:1],
                                scalar2=vec_sb[r0 : r0 + rows, 1:2],
                                op0=mybir.AluOpType.mult,
                                op1=mybir.AluOpType.add,
                            )
                            nc.sync.dma_start(
                                out=out[w, r0 : r0 + rows, gh, c0 : c0 + cw],
                                in_=out_sb[
                                    r0 : r0 + rows, NH * w + gh, c0 : c0 + cw
                                ],
                            )
    nc.compile()
    return nc


def _prep_host(W, b, Sigma):
    """Fold L^{-1} + per-state fp8 scale into conv weights; pack tiles."""
    W64 = W.astype(np.float64)
    b64 = b.astype(np.float64)
    S64 = Sigma.astype(np.float64)
    L = np.linalg.cholesky(S64)
    Li = np.linalg.inv(L)                       # [K, C, C] lower-triangular inv
    logdet = 2.0 * np.sum(np.log(np.diagonal(L, axis1=1, axis2=2)), axis=1)
    W2 = np.einsum("kdc,kcij->kdij", Li, W64)   # [K, C(d), C(ci), 9]
    b2 = np.einsum("kdc,kc->kd", Li, b64)       # [K, C]

    sk = 96.0 / np.abs(W2).max(axis=(1, 2, 3))  # per-state fp8 range scale
    W2s = (W2 * sk[:, None, None, None]).astype(np.float32)
    b2s = (b2 * sk[:, None]).astype(np.float32)

    def kd_cols(a):  # [K, C(d), C(ci)] -> [C(ci), 1024] with col = 64k + d
        return np.ascontiguousarray(np.transpose(a, (2, 0, 1)).reshape(C, 1024))

    wq_np = np.zeros((128, NPAIR, 2, 1024), np.float32)
    for P in range(NPAIR):
        for i in range(2):
            j = 4 * P + 2 * i
            wq_np[0:C, P, i, :] = kd_cols(W2s[:, :, :, j])
            wq_np[C:128, P, i, :] = kd_cols(W2s[:, :, :, j + 1])
    w8_np = np.zeros((128, 1024), np.float32)
    w8_np[0:C, :] = kd_cols(W2s[:, :, :, 8])
    w8_np[C, :] = b2s.reshape(1024)

    const = C * np.log(2.0 * np.pi) + logdet
    vecs_np = np.empty((128, 2), np.float32)
    kk = np.arange(128) % K
    vecs_np[:, 0] = -0.5 / (sk[kk] ** 2)
    vecs_np[:, 1] = -0.5 * const[kk]
    return wq_np.astype(_FP8_NP), w8_np.astype(_FP8_NP), vecs_np


def _run(x, W, b, Sigma, trace=False):
    x = np.asarray(x, np.float32)
    W = np.asarray(W, np.float32)
    b = np.asarray(b, np.float32)
    Sigma = np.asarray(Sigma, np.float32)
    if "nc" not in _CACHE:
        _CACHE["nc"] = _build_program()
    nc = _CACHE["nc"]
    wq_np, w8_np, vecs_np = _prep_host(W, b, Sigma)

    xpad = np.pad(x[0], ((0, 0), (AR, 8)))      # [C, T+16] causal pad + slack
    x8 = xpad.astype(_FP8_NP)                   # quantize once
    ident_np = np.eye(128, dtype=np.float32)
    in_maps = []
    for i in range(NCORES):
        o = TLOC * i
        xq_np = np.empty((128, 2, TLOC + 8), _FP8_NP)
        for p in range(2):
            xq_np[0:C, p, :] = x8[:, o + 2 * p : o + 2 * p + TLOC + 8]
            xq_np[C:128, p, :] = x8[:, o + 2 * p + 1 : o + 2 * p + 1 + TLOC + 8]
        xe_np = np.zeros((128, TLOC), _FP8_NP)
        xe_np[0:C, :] = x8[:, o + 8 : o + 8 + TLOC]
        xe_np[C, :] = np.ones(TLOC, _FP8_NP)
        in_maps.append(
            {
                "xq": xq_np,
                "xeb": xe_np,
                "wq": wq_np,
                "w8": w8_np,
                "ident": ident_np,
                "vecs": vecs_np,
            }
        )
    res = run_bass_kernel_spmd(
        nc, in_maps, core_ids=list(range(NCORES)), trace=trace
    )
    outs = []
    for i in range(NCORES):
        o = res.results[i]["out"]               # [NW, 128, NH, TC]
        o = o.reshape(NW, GRP, K, NH, TC)       # rows -> (c', k)
        o = np.transpose(o, (2, 0, 3, 1, 4)).reshape(K, TLOC)
        outs.append(o)
    full = np.concatenate(outs, axis=1)[None]   # [1, K, T]
    return np.ascontiguousarray(full.astype(np.float32)), res


def kernel(x, W, b, Sigma):
    out, _ = _run(x, W, b, Sigma, trace=bool(int(os.environ.get("BASS_TRACE", "0"))))
    return out
